# revision 1
# baseline (speedup 1.0000x reference)
"""Phi3 attention (B=2, S=2048, HID=3072, H=32, KVH=8, D=96) on 8 trn2
cores. Sharding: core c -> batch b=c//4, head-group g=c%4 (8 q-heads,
2 kv-heads per core). Per core: QKV proj (f32r) -> DRAM spill -> RoPE ->
causal flash-style attention with fused denominator row -> o_proj
partial; host sums the 4 group partials per batch."""
import numpy as np
from contextlib import ExitStack

from concourse import bass, mybir, tile
from concourse.bass_utils import run_bass_kernel_spmd
from concourse.vector_clock import ScopedClock

# ---------------------------------------------------------------------------
# Workaround for this walrus build's limit of 1 sync-wait command per
# instruction: split excess waits onto prefix NoOps (same-engine program
# order preserves semantics), and replace the TileContext exit drain
# (which waits on every semaphore at once) with single-wait nops.
_MAX_WAITS = 1
_ws_counter = [0]


def _split_excess_waits(tc, ordered):
    for _bb, insts in ordered.items():
        new_list = []
        for inst in insts:
            si = inst.sync_info
            waits = list(si.on_wait) if si is not None else []
            if len(waits) > _MAX_WAITS and not isinstance(
                inst, (tile.TileBranchInst, tile.BassTileLoopBlock)
            ):
                excess, keep = waits[:-_MAX_WAITS], waits[-_MAX_WAITS:]
                for j in range(0, len(excess), _MAX_WAITS):
                    chunk = excess[j : j + _MAX_WAITS]
                    _ws_counter[0] += 1
                    nop = mybir.InstNoOp(
                        name=f"I-waitsplit-{_ws_counter[0]}", engine=inst.engine
                    )
                    nop.sync_info = mybir.SyncInfo(on_wait=chunk, on_update=[])
                    tc.nc.register_instruction(nop)
                    new_list.append(nop)
                inst.sync_info = mybir.SyncInfo(
                    on_wait=keep, on_update=list(si.on_update)
                )
            new_list.append(inst)
        insts[:] = new_list


_orig_lower = tile.TileContext._lower_ordered_insts


def _lower_with_split(self, ordered):
    _split_excess_waits(self, ordered)
    return _orig_lower(self, ordered)


def _drain_and_barrier(self, tick_clock, wait_clock):
    vc = tick_clock.global_clock
    for i in range(len(vc)):
        if vc[i] == 0:
            continue
        partial = ScopedClock()
        partial.require_at_least(None, i, vc[i])
        nop = self.nc.sync.nop(hint=f"drain_split_{i}", nofuse=True)
        wait_clock.add_sem_waits(nop.ins, partial)

    drain_inst = self.nc.sync.drain()
    wait_clock.add_sem_waits(drain_inst.ins, ScopedClock())

    self.nc.all_engine_barrier()
    assert self.sems is not None
    popped = self.nc._tile_sem_poison_stack.pop()
    assert popped is self._sem_poison
    self.nc.clear_and_free_semaphores(list(self.sems.allocated().values()))
    self.nc.all_engine_barrier()


def _install_patch():
    tile.TileContext._drain_and_barrier = _drain_and_barrier
    tile.TileContext._lower_ordered_insts = _lower_with_split


_install_patch()
# ---------------------------------------------------------------------------

FP = mybir.dt.float32
FR = mybir.dt.float32r
AF = mybir.ActivationFunctionType
ALU = mybir.AluOpType

H, KVH, D, HID = 32, 8, 96, 3072
B, S = 2, 2048
ROPE_THETA = 10000.0
SCALE = float(D) ** -0.5
NEG = np.float32(-1e9)


def _build():
    nc = bass.Bass()
    hT = nc.declare_dram_parameter("hT", [HID, S], FR, isOutput=False)
    wqkvT = nc.declare_dram_parameter("wqkvT", [HID, 1152], FR, isOutput=False)
    o_wT = nc.declare_dram_parameter("o_wT", [768, HID], FR, isOutput=False)
    cosT_d = nc.declare_dram_parameter("cosT", [96, S], FR, isOutput=False)
    sinT_d = nc.declare_dram_parameter("sinT", [96, S], FR, isOutput=False)
    masks_d = nc.declare_dram_parameter("masks", [128, 4 * 512], FP, isOutput=False)
    mask01_d = nc.declare_dram_parameter("mask01", [128, 4 * 512], FP, isOutput=False)
    neg1_d = nc.declare_dram_parameter("neg1", [1, 96], FR, isOutput=False)
    ident_d = nc.declare_dram_parameter("ident", [96, 96], FP, isOutput=False)
    ones_d = nc.declare_dram_parameter("onescol", [128, 1], FR, isOutput=False)
    outT = nc.declare_dram_parameter("outT", [HID, S], FP, isOutput=True)

    spill = nc.dram_tensor("spill", [1024, S], FR)     # q (0..767) + k (768..959)
    spill_v = nc.dram_tensor("spill_v", [192, S], FP)  # v rows, plain f32

    with tile.TileContext(nc) as tc, ExitStack() as ctx:
        cpool = ctx.enter_context(tc.tile_pool(name="consts", bufs=1))
        mask_t = cpool.tile([128, 4 * 512], FP)
        mask01_t = cpool.tile([128, 4 * 512], FP)
        neg1_t = cpool.tile([1, 96], FR)
        ident_t = cpool.tile([96, 96], FP)
        ones_t = cpool.tile([128, 1], FR)
        nc.sync.dma_start(mask_t[:], masks_d[:])
        nc.sync.dma_start(mask01_t[:], mask01_d[:])
        nc.sync.dma_start(neg1_t[:], neg1_d[:])
        nc.sync.dma_start(ident_t[:], ident_d[:])
        nc.sync.dma_start(ones_t[:], ones_d[:])

        # ------------------ Phase A: QKV projection ------------------
        with tc.tile_pool(name="pa_w", bufs=1) as pw, \
             tc.tile_pool(name="pa_h", bufs=2) as ph, \
             tc.tile_pool(name="pa_ev", bufs=4) as pev, \
             tc.tile_pool(name="pa_ps", bufs=3, space="PSUM") as pps:
            w_k = [pw.tile([128, 1152], FR, name=f"w{k}") for k in range(24)]
            for k in range(24):
                nc.sync.dma_start(w_k[k][:], wqkvT[128 * k:128 * (k + 1), :])
            for n in range(8):           # 256-col chunks of s
                c0 = 256 * n
                ht = [ph.tile([128, 256], FR, name=f"ht{k}") for k in range(24)]
                for k in range(24):
                    nc.sync.dma_start(ht[k][:], hT[128 * k:128 * (k + 1), c0:c0 + 256])
                for m in range(9):
                    ps = pps.tile([128, 256], FP, name="pa_psum")
                    for k in range(24):
                        nc.tensor.matmul(ps[:], lhsT=w_k[k][:, 128 * m:128 * (m + 1)],
                                         rhs=ht[k][:], start=(k == 0), stop=(k == 23))
                    if m < 8:           # q + k rows (and unused v tail of m=7)
                        ev = pev.tile([128, 256], FR, name="pa_ev")
                        nc.scalar.copy(ev[:], ps[:])
                        nc.sync.dma_start(spill[128 * m:128 * (m + 1), c0:c0 + 256], ev[:])
                    if m == 7:          # v rows 960..1023 -> fp spill
                        ev7 = pev.tile([64, 256], FP, name="pa_ev7")
                        nc.scalar.copy(ev7[:], ps[64:128, :])
                        nc.sync.dma_start(spill_v[0:64, c0:c0 + 256], ev7[:])
                    if m == 8:          # v rows 1024..1151 -> fp spill
                        ev8 = pev.tile([128, 256], FP, name="pa_ev8")
                        nc.scalar.copy(ev8[:], ps[:])
                        nc.sync.dma_start(spill_v[64:192, c0:c0 + 256], ev8[:])

        # ------------------ Phase B: RoPE + attention + o_proj ------------------
        with tc.tile_pool(name="qkv", bufs=1) as pqk:
            cos_t = pqk.tile([96, S], FR, name="cos_t")
            sin_t = pqk.tile([96, S], FR, name="sin_t")
            k_h = [pqk.tile([96, S], FR, name=f"k{i}") for i in range(2)]
            vx = [[pqk.tile([128, 97], FR, name=f"vx{i}_{t}") for t in range(16)]
                  for i in range(2)]
            nc.sync.dma_start(cos_t[:], cosT_d[:])
            nc.sync.dma_start(sin_t[:], sinT_d[:])

            # --- B1: k RoPE + v transpose ---
            with tc.tile_pool(name="b1", bufs=1) as pb1, \
                 tc.tile_pool(name="b1ps", bufs=2, space="PSUM") as ptp:
                for i in range(2):
                    dst = k_h[i]
                    nc.sync.dma_start(dst[:], spill[768 + 96 * i:768 + 96 * (i + 1), :])
                    rot = pb1.tile([96, S], FR, name="rope_rot")
                    nc.sync.dma_start(rot[0:48, :], dst[48:96, :])
                    nc.sync.dma_start(rot[48:96, :], dst[0:48, :])
                    nc.vector.tensor_tensor(rot[:], rot[:], sin_t[:], ALU.mult)
                    nc.vector.tensor_tensor(dst[:], dst[:], cos_t[:], ALU.mult)
                    nc.vector.tensor_tensor(dst[:], dst[:], rot[:], ALU.add)
                for i in range(2):
                    vst = pb1.tile([96, S], FP, name="v_stage")
                    nc.sync.dma_start(vst[:], spill_v[96 * i:96 * (i + 1), :])
                    for t in range(16):
                        pst = ptp.tile([128, 96], FP, name="tp_ps")
                        nc.tensor.transpose(pst[:], vst[:, 128 * t:128 * (t + 1)],
                                            ident_t[:])
                        nc.scalar.copy(vx[i][t][:, 0:96], pst[:])
                        nc.sync.dma_start(vx[i][t][:, 96:97], ones_d[:])

            # --- B2: causal attention with per-chunk q RoPE and o_proj
            # interleaved per column chunk (C PE work overlaps ACT/DVE) ---
            with tc.tile_pool(name="b2", bufs=1) as pb2, \
                 tc.tile_pool(name="b2q", bufs=3) as pq, \
                 tc.tile_pool(name="b2an", bufs=2) as pan, \
                 tc.tile_pool(name="b2r", bufs=5) as pbr, \
                 tc.tile_pool(name="pc_w", bufs=1) as pcw, \
                 tc.tile_pool(name="pc_ev", bufs=4) as pce, \
                 tc.tile_pool(name="b2s", bufs=3, space="PSUM") as psc, \
                 tc.tile_pool(name="b2pv", bufs=2, space="PSUM") as ppv, \
                 tc.tile_pool(name="b2bc", bufs=1, space="PSUM") as pbc, \
                 tc.tile_pool(name="pc_ps", bufs=2, space="PSUM") as pcp:
                ow_k = [pcw.tile([128, HID], FR, name=f"ow{k}") for k in range(6)]
                for k in range(6):
                    nc.sync.dma_start(ow_k[k][:], o_wT[128 * k:128 * (k + 1), :])
                for j in range(4):
                    q0 = 512 * j
                    n_kt = 4 * j + 4
                    attnT = [pan.tile([128, 512], FR, name=f"anT{t}")
                             for t in range(6)]
                    for h in range(8):
                        kv = h // 4
                        # load + RoPE this head's q chunk (Pool engine)
                        qc = pq.tile([96, 512], FR, name="qc")
                        qr = pq.tile([96, 512], FR, name="qr")
                        nc.sync.dma_start(qc[:], spill[96 * h:96 * (h + 1), q0:q0 + 512])
                        nc.sync.dma_start(qr[0:48, :], qc[48:96, :])
                        nc.sync.dma_start(qr[48:96, :], qc[0:48, :])
                        nc.gpsimd.tensor_tensor(qr[:], qr[:], sin_t[:, q0:q0 + 512],
                                                ALU.mult)
                        nc.gpsimd.tensor_tensor(qc[:], qc[:], cos_t[:, q0:q0 + 512],
                                                ALU.mult)
                        nc.gpsimd.tensor_tensor(qc[:], qc[:], qr[:], ALU.add)
                        pv = ppv.tile([97, 512], FP, name="pv_ps")
                        # software-pipelined kt loop: PV lags scores/exp by LAG
                        LAG = 4
                        ets = [None] * n_kt
                        for step in range(n_kt + LAG):
                            if step < n_kt:
                                kt = step
                                ss = psc.tile([128, 512], FP, name="s_ps")
                                nc.tensor.matmul(
                                    ss[:], lhsT=k_h[kv][:, 128 * kt:128 * (kt + 1)],
                                    rhs=qc[:], start=True, stop=True)
                                et = pbr.tile([128, 512], FR, name="exp_t")
                                r = kt - 4 * j
                                if r >= 2:      # pre-add -1e9 mask on DVE
                                    pm = pbr.tile([128, 512], FP, name="pm_t")
                                    nc.vector.tensor_tensor(
                                        pm[:], ss[:],
                                        mask_t[:, 512 * r:512 * (r + 1)], ALU.add)
                                    nc.scalar.activation(et[:], pm[:], AF.Exp,
                                                         scale=SCALE)
                                else:
                                    nc.scalar.activation(et[:], ss[:], AF.Exp,
                                                         scale=SCALE)
                                    if r >= 0:  # post-mult 0/1 mask on Pool
                                        nc.gpsimd.tensor_tensor(
                                            et[:], et[:],
                                            mask01_t[:, 512 * r:512 * (r + 1)],
                                            ALU.mult)
                                ets[kt] = et
                            if step >= LAG:
                                kt2 = step - LAG
                                nc.tensor.matmul(
                                    pv[:], lhsT=vx[kv][kt2][:], rhs=ets[kt2][:],
                                    start=(kt2 == 0), stop=(kt2 == n_kt - 1))
                        # one-Newton-refined reciprocal of denominator row
                        r0 = pb2.tile([1, 512], FP, name="nw_r0")
                        e1 = pb2.tile([1, 512], FP, name="nw_e1")
                        n1 = pb2.tile([1, 512], FR, name="nw_n1")
                        nc.vector.reciprocal(r0[:], pv[96:97, :])
                        nc.vector.tensor_tensor(e1[:], pv[96:97, :], r0[:], ALU.mult)
                        nc.vector.scalar_tensor_tensor(
                            n1[:], e1[:], 2.0, r0[:], ALU.subtract, ALU.mult)
                        # broadcast (-n1) across 96 partitions via K=1 matmul
                        bcp = pbc.tile([96, 512], FP, name="bc_ps")
                        nc.tensor.matmul(bcp[:], lhsT=neg1_t[:], rhs=n1[:],
                                         start=True, stop=True)
                        bct = pb2.tile([96, 512], FP, name="bc_t")
                        nc.vector.tensor_copy(bct[:], bcp[:])
                        # normalize into attnT (32-row sections for quadrant rules)
                        for sct in range(3):
                            gl = 96 * h + 32 * sct
                            t, base = gl // 128, gl % 128
                            nc.vector.tensor_tensor(
                                attnT[t][base:base + 32, :],
                                pv[32 * sct:32 * sct + 32, :],
                                bct[32 * sct:32 * sct + 32, :], ALU.mult)
                    # o_proj for this finished column chunk
                    for m in range(24):
                        cps = pcp.tile([128, 512], FP, name="pc_psum")
                        for k in range(6):
                            nc.tensor.matmul(
                                cps[:], lhsT=ow_k[k][:, 128 * m:128 * (m + 1)],
                                rhs=attnT[k][:], start=(k == 0), stop=(k == 5))
                        cev = pce.tile([128, 512], FP, name="pc_ev")
                        if m % 2 == 0:
                            nc.scalar.copy(cev[:], cps[:])
                        else:
                            nc.vector.tensor_copy(cev[:], cps[:])
                        nc.sync.dma_start(outT[128 * m:128 * (m + 1), q0:q0 + 512],
                                          cev[:])

    return nc


_NC_CACHE = [None]


def _host_prep(hidden_states, position_ids, qkv_w, o_w):
    hidden = np.asarray(hidden_states, dtype=np.float32)
    qkv_w = np.asarray(qkv_w, dtype=np.float32)
    o_w = np.asarray(o_w, dtype=np.float32)
    pos = np.asarray(position_ids)

    inv_freq = 1.0 / (ROPE_THETA ** (np.arange(0, D, 2) / D))  # [48]
    masks = np.zeros((128, 4 * 512), dtype=np.float32)
    mask01 = np.zeros((128, 4 * 512), dtype=np.float32)
    kk = np.arange(128)[:, None]
    qq = np.arange(512)[None, :]
    for r in range(4):
        masks[:, 512 * r:512 * (r + 1)] = np.where(128 * r + kk > qq, NEG, 0.0)
        mask01[:, 512 * r:512 * (r + 1)] = np.where(128 * r + kk > qq, 0.0, 1.0)
    neg1 = np.full((1, 96), -1.0, dtype=np.float32)
    ident = np.eye(96, dtype=np.float32)
    onescol = np.ones((128, 1), dtype=np.float32)

    hT = [np.ascontiguousarray(hidden[b].T) for b in range(B)]
    cosT, sinT = [], []
    for b in range(B):
        ang = pos[b].astype(np.float64)[None, :] * inv_freq[:, None]  # [48,S]
        ang = np.concatenate([ang, ang], 0)                           # [96,S]
        cosT.append(np.cos(ang).astype(np.float32))
        st = np.sin(ang).astype(np.float32)
        st[0:48] *= -1.0
        sinT.append(st)

    in_maps = []
    for c in range(8):
        b, g = c // 4, c % 4
        q_rows = qkv_w[768 * g:768 * (g + 1)]
        k_rows = qkv_w[3072 + 192 * g:3072 + 192 * (g + 1)]
        v_rows = qkv_w[3840 + 192 * g:3840 + 192 * (g + 1)]
        wqkvT = np.ascontiguousarray(
            np.concatenate([q_rows, k_rows, v_rows], 0).T)            # [3072,1152]
        o_wT = np.ascontiguousarray(o_w[:, 768 * g:768 * (g + 1)].T)  # [768,3072]
        in_maps.append({
            "hT": hT[b], "wqkvT": wqkvT, "o_wT": o_wT,
            "cosT": cosT[b], "sinT": sinT[b],
            "masks": masks, "mask01": mask01, "neg1": neg1, "ident": ident,
            "onescol": onescol,
        })
    return in_maps


def kernel(**inputs):
    in_maps = _host_prep(**inputs)
    if _NC_CACHE[0] is None:
        _NC_CACHE[0] = _build()
    res = run_bass_kernel_spmd(_NC_CACHE[0], in_maps, list(range(8)))
    out = np.empty((B, S, HID), dtype=np.float32)
    for b in range(B):
        acc = res.results[4 * b]["outT"]
        for g in range(1, 4):
            acc = acc + res.results[4 * b + g]["outT"]
        out[b] = acc.T
    return out



# revision 17
# speedup vs baseline: 1.3356x; 1.3356x over previous
"""Phi3 attention (B=2, S=2048, HID=3072, H=32, KVH=8, D=96) on 8 trn2
cores. Sharding: core c -> batch b=c//4, head-group g=c%4 (8 q-heads,
2 kv-heads per core). v2: all-bf16 operands, SBUF-resident q/k/v (no
DRAM spill), per-chunk RoPE pipelined into the QKV projection, xbar
DMA v-transpose, causally width-trimmed scores/PV, divide-based
softmax normalization with Pool partition-broadcast, o_proj interleaved
into the next chunk's head loop, direct PSUM->DRAM output DMA; host
sums the 4 group partials per batch."""
import numpy as np
import ml_dtypes
from contextlib import ExitStack

from concourse import bass, mybir, tile
from concourse.bass_utils import run_bass_kernel_spmd
from concourse.vector_clock import ScopedClock

# ---------------------------------------------------------------------------
# Workaround for this walrus build's limit of 1 sync-wait command per
# instruction: split excess waits onto prefix NoOps (same-engine program
# order preserves semantics), and replace the TileContext exit drain
# (which waits on every semaphore at once) with single-wait nops.
_MAX_WAITS = 1
_ws_counter = [0]


def _split_excess_waits(tc, ordered):
    for _bb, insts in ordered.items():
        new_list = []
        for inst in insts:
            si = inst.sync_info
            waits = list(si.on_wait) if si is not None else []
            if len(waits) > _MAX_WAITS and not isinstance(
                inst, (tile.TileBranchInst, tile.BassTileLoopBlock)
            ):
                excess, keep = waits[:-_MAX_WAITS], waits[-_MAX_WAITS:]
                for j in range(0, len(excess), _MAX_WAITS):
                    chunk = excess[j : j + _MAX_WAITS]
                    _ws_counter[0] += 1
                    nop = mybir.InstNoOp(
                        name=f"I-waitsplit-{_ws_counter[0]}", engine=inst.engine
                    )
                    nop.sync_info = mybir.SyncInfo(on_wait=chunk, on_update=[])
                    tc.nc.register_instruction(nop)
                    new_list.append(nop)
                inst.sync_info = mybir.SyncInfo(
                    on_wait=keep, on_update=list(si.on_update)
                )
            new_list.append(inst)
        insts[:] = new_list


_orig_lower = tile.TileContext._lower_ordered_insts


def _lower_with_split(self, ordered):
    _split_excess_waits(self, ordered)
    return _orig_lower(self, ordered)


def _drain_and_barrier(self, tick_clock, wait_clock):
    vc = tick_clock.global_clock
    for i in range(len(vc)):
        if vc[i] == 0:
            continue
        partial = ScopedClock()
        partial.require_at_least(None, i, vc[i])
        nop = self.nc.sync.nop(hint=f"drain_split_{i}", nofuse=True)
        wait_clock.add_sem_waits(nop.ins, partial)

    drain_inst = self.nc.sync.drain()
    wait_clock.add_sem_waits(drain_inst.ins, ScopedClock())

    self.nc.all_engine_barrier()
    assert self.sems is not None
    popped = self.nc._tile_sem_poison_stack.pop()
    assert popped is self._sem_poison
    self.nc.clear_and_free_semaphores(list(self.sems.allocated().values()))
    self.nc.all_engine_barrier()


def _install_patch():
    tile.TileContext._drain_and_barrier = _drain_and_barrier
    tile.TileContext._lower_ordered_insts = _lower_with_split


_install_patch()
# ---------------------------------------------------------------------------

FP = mybir.dt.float32
FR = mybir.dt.float32r
BF = mybir.dt.bfloat16
AF = mybir.ActivationFunctionType
ALU = mybir.AluOpType

H, KVH, D, HID = 32, 8, 96, 3072
B, S = 2, 2048
ROPE_THETA = 10000.0
SCALE = float(D) ** -0.5
NQ = 8      # q heads per core
NKV = 2     # kv heads per core
LAG = 4


def _qkv_evac_pieces(m):
    """Evacuation pieces of phase-A psum m-tile m (rows 128m..128m+128 of
    the 1152 qkv rows) -> list of (psum_r0, psum_r1, dest_kind, dest_idx,
    dest_r0). dest_kind: 0=q head, 1=k head, 2=v head."""
    out = []
    lo, hi = 128 * m, 128 * (m + 1)
    bounds = [(0, 768, 0, 96), (768, 960, 1, 96), (960, 1152, 2, 96)]
    r = lo
    while r < hi:
        for s0, s1, kind, hd in bounds:
            if s0 <= r < s1:
                idx = (r - s0) // hd
                within = (r - s0) % hd
                span = min(hi - r, hd - within, s1 - r)
                out.append((r - lo, r - lo + span, kind, idx, within))
                r += span
                break
    return out


def _quad_split(a, b, span):
    """Split [0, span) into offsets valid for partition ranges starting at
    a+off and b+off: a range starting at partition p may span at most its
    buddy block (128 at 0, 64 at 64, 32 at 32/96)."""
    def blk(x):
        x %= 128
        return 128 if x == 0 else (x & -x)
    out = []
    p = 0
    while p < span:
        step = min(blk(a + p), blk(b + p), span - p)
        out.append((p, p + step))
        p += step
    return out


def _norm_pieces(h):
    """Normalize pieces for head h: (pv_r0, pv_r1, attnT_tile, attnT_r0)."""
    out = []
    g0 = 96 * h
    r = 0
    while r < 96:
        t, within = (g0 + r) // 128, (g0 + r) % 128
        span = min(96 - r, 128 - within)
        out.append((r, r + span, t, within))
        r += span
    return out


def _build():
    nc = bass.Bass()
    hT = nc.declare_dram_parameter("hT", [HID, S], BF, isOutput=False)
    wqkvT = nc.declare_dram_parameter("wqkvT", [HID, 1152], BF, isOutput=False)
    o_wT = nc.declare_dram_parameter("o_wT", [768, HID], BF, isOutput=False)
    cosT_d = nc.declare_dram_parameter("cosT", [96, S], BF, isOutput=False)
    sinT_d = nc.declare_dram_parameter("sinT", [96, S], BF, isOutput=False)
    tri_d = nc.declare_dram_parameter("tri", [128, 128], BF, isOutput=False)
    ones_d = nc.declare_dram_parameter("ones96", [1, 96], FR, isOutput=False)
    outT = nc.declare_dram_parameter("outT", [HID, S], BF, isOutput=True)

    with tile.TileContext(nc) as tc, ExitStack() as ctx:
        pers = ctx.enter_context(tc.tile_pool(name="pers", bufs=1))
        tri_t = pers.tile([128, 128], BF)
        cos_t = pers.tile([96, S], BF)
        sin_t = pers.tile([96, S], BF)
        q_t = [pers.tile([96, S], BF, name=f"q{h}") for h in range(NQ)]
        k_t = [pers.tile([96, S], BF, name=f"k{i}") for i in range(NKV)]
        vx_t = [pers.tile([128, 16 * 97], BF, name=f"vx{i}") for i in range(NKV)]
        ones_t = pers.tile([1, 96], FR, name="ones96")
        nc.sync.dma_start(tri_t[:], tri_d[:])
        nc.sync.dma_start(cos_t[:], cosT_d[:])
        nc.sync.dma_start(sin_t[:], sinT_d[:])
        nc.sync.dma_start(ones_t[:], ones_d[:])
        for i in range(NKV):
            nc.gpsimd.memset(vx_t[i][:], 1.0)

        # ------------------ Phase A: QKV projection + RoPE + vT ------------
        rr = [0]  # round-robin engine counter for elementwise work

        def _copy(eng, dst, src):
            # psum-reading copies: ACT or DVE only (GPSIMD cannot access PSUM)
            if eng % 2 == 0:
                nc.scalar.copy(dst, src)
            else:
                nc.vector.tensor_copy(dst, src)

        with tc.tile_pool(name="pa_w", bufs=1) as pw, \
             tc.tile_pool(name="pa_h", bufs=2) as ph, \
             tc.tile_pool(name="pa_rot", bufs=2) as prot, \
             tc.tile_pool(name="pa_vs", bufs=2) as pvs, \
             tc.tile_pool(name="pa_vt", bufs=3) as pvt, \
             tc.tile_pool(name="pa_ps", bufs=5, space="PSUM") as pps:
            w_k = [pw.tile([128, 1152], BF, name=f"w{k}") for k in range(24)]
            for n in range(4):
                c0 = 512 * n
                ht = [ph.tile([128, 512], BF, name=f"ht{k}") for k in range(24)]
                for k in range(24):
                    if n == 0:
                        nc.sync.dma_start(w_k[k][:], wqkvT[128 * k:128 * (k + 1), :])
                    nc.sync.dma_start(ht[k][:], hT[128 * k:128 * (k + 1), c0:c0 + 512])
                vst = [pvs.tile([96, 512], BF, name=f"vs{i}") for i in range(NKV)]
                for mlo, mhi in [(0, 4), (4, 9)]:
                    pss = {m: pps.tile([128, 512], FP, name="pa_ps") for m in range(mlo, mhi)}
                    for k in range(24):
                        for m in range(mlo, mhi):
                            nc.tensor.matmul(
                                pss[m][:], lhsT=w_k[k][:, 128 * m:128 * (m + 1)],
                                rhs=ht[k][:], start=(k == 0), stop=(k == 23))
                    for m in range(mlo, mhi):
                        for r0, r1, kind, idx, d0 in _qkv_evac_pieces(m):
                            tgt = (q_t, k_t, None)[kind]
                            for p0, p1 in _quad_split(r0, d0, r1 - r0):
                                if kind == 2:
                                    dst = vst[idx][d0 + p0:d0 + p1, 0:512]
                                else:
                                    dst = tgt[idx][d0 + p0:d0 + p1, c0:c0 + 512]
                                _copy(rr[0], dst, pss[m][r0 + p0:r0 + p1, :])
                                rr[0] += 1
                # RoPE on this chunk's q/k columns
                for t in range(NQ + NKV):
                    src = q_t[t] if t < NQ else k_t[t - NQ]
                    rot = prot.tile([96, 512], BF, name="rot")
                    nc.sync.dma_start(rot[0:48, :], src[48:96, c0:c0 + 512])
                    nc.sync.dma_start(rot[48:96, :], src[0:48, c0:c0 + 512])
                    # rot *= sin ; src *= cos ; src += rot  (spread engines)
                    ee = [nc.vector, nc.gpsimd, nc.vector][t % 3]
                    ee.tensor_tensor(rot[:], rot[:], sin_t[:, c0:c0 + 512], ALU.mult)
                    ee.tensor_tensor(src[:, c0:c0 + 512], src[:, c0:c0 + 512],
                                     cos_t[:, c0:c0 + 512], ALU.mult)
                    ee.tensor_tensor(src[:, c0:c0 + 512], src[:, c0:c0 + 512],
                                     rot[:], ALU.add)
                # v transpose: [96,128] -> [128,96] via xbar DMA, then copy
                for i in range(NKV):
                    for t in range(4):
                        kt = 4 * n + t
                        vt = pvt.tile([128, 96], BF, name="vt")
                        nc.sync.dma_start_transpose(vt[:], vst[i][:, 128 * t:128 * (t + 1)])
                        nc.gpsimd.tensor_copy(vx_t[i][:, 97 * kt:97 * kt + 96], vt[:])

        # ------------------ Phase B: attention + o_proj --------------------
        with tc.tile_pool(name="pb_ow", bufs=1) as pow_, \
             tc.tile_pool(name="pb_an", bufs=2) as pan, \
             tc.tile_pool(name="pb_et", bufs=8) as pet, \
             tc.tile_pool(name="pb_d", bufs=3) as pbd, \
             tc.tile_pool(name="pb_ce", bufs=3) as pce, \
             tc.tile_pool(name="pb_ss", bufs=3, space="PSUM") as psc, \
             tc.tile_pool(name="pb_pv", bufs=2, space="PSUM") as ppv, \
             tc.tile_pool(name="pb_bc", bufs=1, space="PSUM") as pbc, \
             tc.tile_pool(name="pb_op", bufs=2, space="PSUM") as pcp:
            ow_k = [pow_.tile([128, HID], BF, name=f"ow{k}") for k in range(6)]
            for k in range(6):
                nc.sync.dma_start(ow_k[k][:], o_wT[128 * k:128 * (k + 1), :])

            attnT_prev = None

            def emit_oproj(jj, m):
                cps = pcp.tile([128, 512], FP, name="op_ps")
                for k in range(6):
                    nc.tensor.matmul(
                        cps[:], lhsT=ow_k[k][:, 128 * m:128 * (m + 1)],
                        rhs=attnT_prev[k][:], start=(k == 0), stop=(k == 5))
                cev = pce.tile([128, 512], BF, name="op_ev")
                _copy(m, cev[:], cps[:])
                nc.sync.dma_start(outT[128 * m:128 * (m + 1), 512 * jj:512 * (jj + 1)],
                                  cev[:])

            for j in range(4):
                q0 = 512 * j
                n_kt = 4 * j + 4
                attnT = [pan.tile([128, 512], BF, name=f"anT{t}") for t in range(6)]
                for h in range(NQ):
                    kv = h // 4
                    pv = ppv.tile([97, 512], FP, name="pv_ps")
                    ets = [None] * n_kt
                    for step in range(n_kt + LAG):
                        if step < n_kt:
                            kt = step
                            r = kt - 4 * j
                            off = 128 * r if r > 0 else 0
                            w = 512 - off
                            ss = psc.tile([128, 512], FP, name="ss_ps")
                            nc.tensor.matmul(
                                ss[:, 0:w], lhsT=k_t[kv][:, 128 * kt:128 * (kt + 1)],
                                rhs=q_t[h][:, q0 + off:q0 + 512],
                                start=True, stop=True)
                            et = pet.tile([128, 512], BF, name="et")
                            nc.scalar.activation(et[:, 0:w], ss[:, 0:w], AF.Exp,
                                                 scale=SCALE)
                            if r >= 0:  # zero the causal triangle
                                eng = nc.vector if (kt % 2 == 0) else nc.gpsimd
                                eng.tensor_tensor(et[:, 0:128], et[:, 0:128],
                                                  tri_t[:], ALU.mult)
                            ets[kt] = (et, off, w)
                        if step >= LAG:
                            kt2 = step - LAG
                            et, off, w = ets[kt2]
                            nc.tensor.matmul(
                                pv[:, off:off + w],
                                lhsT=vx_t[kv][:, 97 * kt2:97 * (kt2 + 1)],
                                rhs=et[:, 0:w], start=(kt2 == 0),
                                stop=(kt2 == n_kt - 1), skip_group_check=True)
                    # normalization: recip(den) -> PE broadcast -> mult into attnT
                    rec = pbd.tile([1, 512], FR, name="rec")
                    bct = pbd.tile([96, 512], FP, name="bct")
                    with nc.allow_low_precision(reason="f32r stores full f32 bits"):
                        nc.vector.reciprocal(rec[:], pv[96:97, :])
                    bcp = pbc.tile([96, 512], FP, name="bc_ps")
                    nc.tensor.matmul(bcp[:], lhsT=ones_t[:], rhs=rec[:],
                                     start=True, stop=True)
                    nc.scalar.copy(bct[:], bcp[:])
                    for r0, r1, t, a0 in _norm_pieces(h):
                        for p0, p1 in _quad_split(r0, a0, r1 - r0):
                            nc.vector.tensor_tensor(
                                attnT[t][a0 + p0:a0 + p1, :], pv[r0 + p0:r0 + p1, :],
                                bct[r0 + p0:r0 + p1, :], ALU.mult)
                    # interleave o_proj of the previous chunk
                    if attnT_prev is not None:
                        for m in range(3 * h, 3 * h + 3):
                            emit_oproj(j - 1, m)
                attnT_prev = attnT
            for m in range(24):
                emit_oproj(3, m)

    return nc


_NC_CACHE = [None]


def _host_prep(hidden_states, position_ids, qkv_w, o_w):
    BFn = ml_dtypes.bfloat16
    hidden = np.asarray(hidden_states, dtype=np.float32)
    qkv_w = np.asarray(qkv_w, dtype=np.float32)
    o_w = np.asarray(o_w, dtype=np.float32)
    pos = np.asarray(position_ids)

    inv_freq = 1.0 / (ROPE_THETA ** (np.arange(0, D, 2) / D))  # [48]
    kk = np.arange(128)[:, None]
    uu = np.arange(128)[None, :]
    tri = (kk <= uu).astype(BFn)

    hT = [np.ascontiguousarray(hidden[b].T).astype(BFn) for b in range(B)]
    cosT, sinT = [], []
    for b in range(B):
        ang = pos[b].astype(np.float64)[None, :] * inv_freq[:, None]  # [48,S]
        ang = np.concatenate([ang, ang], 0)                           # [96,S]
        cosT.append(np.cos(ang).astype(BFn))
        st = np.sin(ang).astype(np.float64)
        st[0:48] *= -1.0
        sinT.append(st.astype(BFn))

    in_maps = []
    for c in range(8):
        b, g = c // 4, c % 4
        q_rows = qkv_w[768 * g:768 * (g + 1)]
        k_rows = qkv_w[3072 + 192 * g:3072 + 192 * (g + 1)]
        v_rows = qkv_w[3840 + 192 * g:3840 + 192 * (g + 1)]
        wqkvT = np.ascontiguousarray(
            np.concatenate([q_rows, k_rows, v_rows], 0).T).astype(BFn)
        o_wT = np.ascontiguousarray(o_w[:, 768 * g:768 * (g + 1)].T).astype(BFn)
        in_maps.append({
            "hT": hT[b], "wqkvT": wqkvT, "o_wT": o_wT,
            "cosT": cosT[b], "sinT": sinT[b], "tri": tri,
            "ones96": np.ones((1, 96), dtype=np.float32),
        })
    return in_maps


def kernel(**inputs):
    in_maps = _host_prep(**inputs)
    if _NC_CACHE[0] is None:
        _NC_CACHE[0] = _build()
    res = run_bass_kernel_spmd(_NC_CACHE[0], in_maps, list(range(8)))
    out = np.empty((B, S, HID), dtype=np.float32)
    for b in range(B):
        acc = res.results[4 * b]["outT"].astype(np.float32)
        for g in range(1, 4):
            acc = acc + res.results[4 * b + g]["outT"].astype(np.float32)
        out[b] = acc.T
    return out


# revision 30
# speedup vs baseline: 1.3929x; 1.0429x over previous
"""Phi3 attention (B=2, S=2048, HID=3072, H=32, KVH=8, D=96) on 8 trn2
cores. Sharding: core c -> batch b=c//4, head-group g=c%4 (8 q-heads,
2 kv-heads per core). v2: all-bf16 operands, SBUF-resident q/k/v (no
DRAM spill), per-chunk RoPE pipelined into the QKV projection, xbar
DMA v-transpose, causally width-trimmed scores/PV, divide-based
softmax normalization with Pool partition-broadcast, o_proj interleaved
into the next chunk's head loop, direct PSUM->DRAM output DMA; host
sums the 4 group partials per batch."""
import numpy as np
import ml_dtypes
from contextlib import ExitStack

from concourse import bass, mybir, tile
from concourse.bass_utils import run_bass_kernel_spmd
from concourse.vector_clock import ScopedClock

# ---------------------------------------------------------------------------
# Workaround for this walrus build's limit of 1 sync-wait command per
# instruction: split excess waits onto prefix NoOps (same-engine program
# order preserves semantics), and replace the TileContext exit drain
# (which waits on every semaphore at once) with single-wait nops.
_MAX_WAITS = 1
_ws_counter = [0]


def _split_excess_waits(tc, ordered):
    for _bb, insts in ordered.items():
        new_list = []
        for inst in insts:
            si = inst.sync_info
            waits = list(si.on_wait) if si is not None else []
            if len(waits) > _MAX_WAITS and not isinstance(
                inst, (tile.TileBranchInst, tile.BassTileLoopBlock)
            ):
                excess, keep = waits[:-_MAX_WAITS], waits[-_MAX_WAITS:]
                for j in range(0, len(excess), _MAX_WAITS):
                    chunk = excess[j : j + _MAX_WAITS]
                    _ws_counter[0] += 1
                    nop = mybir.InstNoOp(
                        name=f"I-waitsplit-{_ws_counter[0]}", engine=inst.engine
                    )
                    nop.sync_info = mybir.SyncInfo(on_wait=chunk, on_update=[])
                    tc.nc.register_instruction(nop)
                    new_list.append(nop)
                inst.sync_info = mybir.SyncInfo(
                    on_wait=keep, on_update=list(si.on_update)
                )
            new_list.append(inst)
        insts[:] = new_list


_orig_lower = tile.TileContext._lower_ordered_insts


def _lower_with_split(self, ordered):
    _split_excess_waits(self, ordered)
    return _orig_lower(self, ordered)


def _drain_and_barrier(self, tick_clock, wait_clock):
    vc = tick_clock.global_clock
    for i in range(len(vc)):
        if vc[i] == 0:
            continue
        partial = ScopedClock()
        partial.require_at_least(None, i, vc[i])
        nop = self.nc.sync.nop(hint=f"drain_split_{i}", nofuse=True)
        wait_clock.add_sem_waits(nop.ins, partial)

    drain_inst = self.nc.sync.drain()
    wait_clock.add_sem_waits(drain_inst.ins, ScopedClock())

    self.nc.all_engine_barrier()
    assert self.sems is not None
    popped = self.nc._tile_sem_poison_stack.pop()
    assert popped is self._sem_poison
    self.nc.clear_and_free_semaphores(list(self.sems.allocated().values()))
    self.nc.all_engine_barrier()


def _install_patch():
    tile.TileContext._drain_and_barrier = _drain_and_barrier
    tile.TileContext._lower_ordered_insts = _lower_with_split


_install_patch()
# ---------------------------------------------------------------------------

FP = mybir.dt.float32
FR = mybir.dt.float32r
BF = mybir.dt.bfloat16
AF = mybir.ActivationFunctionType
ALU = mybir.AluOpType

H, KVH, D, HID = 32, 8, 96, 3072
B, S = 2, 2048
ROPE_THETA = 10000.0
SCALE = float(D) ** -0.5
NQ = 8      # q heads per core
NKV = 2     # kv heads per core
LAG = 6


def _qkv_evac_pieces(m):
    """Evacuation pieces of phase-A psum m-tile m (rows 128m..128m+128 of
    the 1152 qkv rows) -> list of (psum_r0, psum_r1, dest_kind, dest_idx,
    dest_r0). dest_kind: 0=q head, 1=k head, 2=v head."""
    out = []
    lo, hi = 128 * m, 128 * (m + 1)
    bounds = [(0, 768, 0, 96), (768, 960, 1, 96), (960, 1152, 2, 96)]
    r = lo
    while r < hi:
        for s0, s1, kind, hd in bounds:
            if s0 <= r < s1:
                idx = (r - s0) // hd
                within = (r - s0) % hd
                span = min(hi - r, hd - within, s1 - r)
                out.append((r - lo, r - lo + span, kind, idx, within))
                r += span
                break
    return out


def _quad_split(a, b, span):
    """Split [0, span) into offsets valid for partition ranges starting at
    a+off and b+off: a range starting at partition p may span at most its
    buddy block (128 at 0, 64 at 64, 32 at 32/96)."""
    def blk(x):
        x %= 128
        return 128 if x == 0 else (x & -x)
    out = []
    p = 0
    while p < span:
        step = min(blk(a + p), blk(b + p), span - p)
        out.append((p, p + step))
        p += step
    return out


def _norm_pieces(h):
    """Normalize pieces for head h: (pv_r0, pv_r1, attnT_tile, attnT_r0)."""
    out = []
    g0 = 96 * h
    r = 0
    while r < 96:
        t, within = (g0 + r) // 128, (g0 + r) % 128
        span = min(96 - r, 128 - within)
        out.append((r, r + span, t, within))
        r += span
    return out


def _build():
    nc = bass.Bass()
    hT = nc.declare_dram_parameter("hT", [HID, S], BF, isOutput=False)
    wqkvT = nc.declare_dram_parameter("wqkvT", [HID, 1152], BF, isOutput=False)
    o_wT = nc.declare_dram_parameter("o_wT", [768, HID], BF, isOutput=False)
    cosT_d = nc.declare_dram_parameter("cosT", [96, S], BF, isOutput=False)
    sinT_d = nc.declare_dram_parameter("sinT", [96, S], BF, isOutput=False)
    tri_d = nc.declare_dram_parameter("tri", [128, 128], BF, isOutput=False)
    ones_d = nc.declare_dram_parameter("ones96", [1, 96], FR, isOutput=False)
    outT = nc.declare_dram_parameter("outT", [HID, S], BF, isOutput=True)

    with tile.TileContext(nc) as tc, ExitStack() as ctx:
        pers = ctx.enter_context(tc.tile_pool(name="pers", bufs=1))
        tri_t = pers.tile([128, 128], BF)
        cos_t = pers.tile([96, S], BF)
        sin_t = pers.tile([96, S], BF)
        q_t = [pers.tile([96, S], BF, name=f"q{h}") for h in range(NQ)]
        k_t = [pers.tile([96, S], BF, name=f"k{i}") for i in range(NKV)]
        vx_t = [pers.tile([128, 16 * 97], BF, name=f"vx{i}") for i in range(NKV)]
        ones_t = pers.tile([1, 96], FR, name="ones96")
        for i in range(NKV):
            nc.gpsimd.memset(vx_t[i][:], 1.0)

        # ------------------ interleaved QKV projection + attention ---------
        rr = [0]  # round-robin engine counter for psum-evac copies

        def _copy(eng, dst, src):
            # psum-reading copies: ACT or DVE only (GPSIMD cannot access PSUM)
            if eng % 2 == 0:
                nc.scalar.copy(dst, src)
            else:
                nc.vector.tensor_copy(dst, src)

        attnTs = {}

        # ---------------- Phase A: QKV projection + RoPE + vT --------------
        with tc.tile_pool(name="pa_w", bufs=1) as pw, \
             tc.tile_pool(name="pa_h", bufs=2) as ph, \
             tc.tile_pool(name="pa_rot", bufs=5) as prot, \
             tc.tile_pool(name="pa_vs", bufs=2) as pvs, \
             tc.tile_pool(name="pa_vt", bufs=3) as pvt, \
             tc.tile_pool(name="pa_ps", bufs=5, space="PSUM") as pps:
            w_k = [pw.tile([128, 1152], BF, name=f"w{k}") for k in range(24)]

            def emit_a_dma(n, with_w=False):
                c0 = 512 * n
                ht = [ph.tile([128, 512], BF, name=f"ht{k}") for k in range(24)]
                for k in range(24):
                    if with_w:
                        nc.sync.dma_start(w_k[k][:],
                                          wqkvT[128 * k:128 * (k + 1), :])
                    nc.sync.dma_start(ht[k][:],
                                      hT[128 * k:128 * (k + 1), c0:c0 + 512])
                vst = [pvs.tile([96, 512], BF, name=f"vs{i}")
                       for i in range(NKV)]
                return ht, vst

            def a_mtiles(n, ht, vst, group):
                c0 = 512 * n
                pss = {m: pps.tile([128, 512], FP, name="pa_ps")
                       for m in group}
                for k in range(24):
                    for m in group:
                        nc.tensor.matmul(
                            pss[m][:], lhsT=w_k[k][:, 128 * m:128 * (m + 1)],
                            rhs=ht[k][:], start=(k == 0), stop=(k == 23))
                for m in group:
                    for r0, r1, kind, idx, d0 in _qkv_evac_pieces(m):
                        tgt = (q_t, k_t, None)[kind]
                        for p0, p1 in _quad_split(r0, d0, r1 - r0):
                            if kind == 2:
                                dst = vst[idx][d0 + p0:d0 + p1, 0:512]
                            else:
                                dst = tgt[idx][d0 + p0:d0 + p1, c0:c0 + 512]
                            nc.scalar.copy(dst, pss[m][r0 + p0:r0 + p1, :])

            def a_rope_vt(n, vst):
                # rot staging via the ACT DMA queue so the bulk SP queue is
                # never gated by the rope dependency chain; k tensors first
                c0 = 512 * n
                for t in [NQ, NQ + 1] + list(range(NQ)):
                    src = q_t[t] if t < NQ else k_t[t - NQ]
                    rot = prot.tile([96, 512], BF, name="rot")
                    nc.sync.dma_start(rot[0:48, :], src[48:96, c0:c0 + 512])
                    nc.sync.dma_start(rot[48:96, :], src[0:48, c0:c0 + 512])
                    nc.vector.tensor_tensor(rot[:], rot[:],
                                            sin_t[:, c0:c0 + 512], ALU.mult)
                    nc.vector.tensor_tensor(src[:, c0:c0 + 512],
                                            src[:, c0:c0 + 512],
                                            cos_t[:, c0:c0 + 512], ALU.mult)
                    nc.vector.tensor_tensor(src[:, c0:c0 + 512],
                                            src[:, c0:c0 + 512], rot[:], ALU.add)
                for i in range(NKV):
                    for t in range(4):
                        kt = 4 * n + t
                        vt = pvt.tile([128, 96], BF, name="vt")
                        nc.sync.dma_start_transpose(
                            vt[:], vst[i][:, 128 * t:128 * (t + 1)])
                        nc.gpsimd.tensor_copy(
                            vx_t[i][:, 97 * kt:97 * kt + 96], vt[:])

            ht0, vst0 = emit_a_dma(0, with_w=True)
            nc.sync.dma_start(tri_t[:], tri_d[:])
            nc.sync.dma_start(cos_t[:], cosT_d[:])
            nc.sync.dma_start(sin_t[:], sinT_d[:])
            nc.sync.dma_start(ones_t[:], ones_d[:])
            ht1, vst1 = emit_a_dma(1)
            GROUPS = ([0, 2], [2, 4], [4, 6], [6, 8], [8, 9])
            for group in GROUPS:
                a_mtiles(0, ht0, vst0, range(*group))
            ht2, vst2 = emit_a_dma(2)
            a_rope_vt(0, vst0)
            for group in GROUPS:
                a_mtiles(1, ht1, vst1, range(*group))
            ht3, vst3 = emit_a_dma(3)
            a_rope_vt(1, vst1)
            for group in GROUPS:
                a_mtiles(2, ht2, vst2, range(*group))
            a_rope_vt(2, vst2)
            for group in GROUPS:
                a_mtiles(3, ht3, vst3, range(*group))
            a_rope_vt(3, vst3)

        # ---------------- Phase B: attention + o_proj ----------------------
        with tc.tile_pool(name="pb_an", bufs=2) as pan, \
             tc.tile_pool(name="pb_et", bufs=8) as pet, \
             tc.tile_pool(name="pb_d", bufs=3) as pbd, \
             tc.tile_pool(name="pb_ow", bufs=1) as pow_, \
             tc.tile_pool(name="pb_ce", bufs=3) as pce, \
             tc.tile_pool(name="pb_ss", bufs=3, space="PSUM") as psc, \
             tc.tile_pool(name="pb_pv", bufs=2, space="PSUM") as ppv, \
             tc.tile_pool(name="pb_bc", bufs=1, space="PSUM") as pbc, \
             tc.tile_pool(name="pb_op", bufs=2, space="PSUM") as pcp:
            ow_k = [pow_.tile([128, HID], BF, name=f"ow{k}") for k in range(6)]
            for k in range(6):
                nc.sync.dma_start(ow_k[k][:], o_wT[128 * k:128 * (k + 1), :])

            def emit_attn_chunk(j, backfill):
                """Emit attention chunk j; pull PE backfill thunks from the
                iterator at pipeline bubbles."""
                def pump(k=1):
                    for _ in range(k):
                        t = next(backfill, None)
                        if t is None:
                            return False
                        t()
                    return True

                q0 = 512 * j
                n_kt = 4 * j + 4
                attnT = [pan.tile([128, 512], BF, name=f"anT{t}") for t in range(6)]
                attnTs[j] = attnT
                for h in range(NQ):
                    kv = h // 4
                    pv = ppv.tile([97, 512], FP, name="pv_ps")
                    ets = [None] * n_kt
                    for step in range(n_kt + LAG):
                        if step < n_kt:
                            kt = step
                            r = kt - 4 * j
                            off = 128 * r if r > 0 else 0
                            w = 512 - off
                            ss = psc.tile([128, 512], FP, name="ss_ps")
                            nc.tensor.matmul(
                                ss[:, 0:w], lhsT=k_t[kv][:, 128 * kt:128 * (kt + 1)],
                                rhs=q_t[h][:, q0 + off:q0 + 512],
                                start=True, stop=True)
                            et = pet.tile([128, 512], BF, name="et")
                            nc.scalar.activation(et[:, 0:w], ss[:, 0:w], AF.Exp,
                                                 scale=SCALE)
                            if r >= 0:  # zero the causal triangle
                                nc.vector.tensor_tensor(et[:, 0:128], et[:, 0:128],
                                                        tri_t[:], ALU.mult)
                            ets[kt] = (et, off, w)
                        if step >= LAG:
                            kt2 = step - LAG
                            et, off, w = ets[kt2]
                            nc.tensor.matmul(
                                pv[:, off:off + w],
                                lhsT=vx_t[kv][:, 97 * kt2:97 * (kt2 + 1)],
                                rhs=et[:, 0:w], start=(kt2 == 0),
                                stop=(kt2 == n_kt - 1), skip_group_check=True)
                        if step == 3 or (step > 3 and (step - 3) % 4 == 0
                                         and step < n_kt):
                            pump()
                    # normalization: recip(den) -> PE broadcast -> mult
                    rec = pbd.tile([1, 512], FR, name="rec")
                    bct = pbd.tile([96, 512], FP, name="bct")
                    with nc.allow_low_precision(reason="f32r stores full f32 bits"):
                        nc.vector.reciprocal(rec[:], pv[96:97, :])
                    bcp = pbc.tile([96, 512], FP, name="bc_ps")
                    nc.tensor.matmul(bcp[:], lhsT=ones_t[:], rhs=rec[:],
                                     start=True, stop=True)
                    nc.scalar.copy(bct[:], bcp[:])
                    for r0, r1, t, a0 in _norm_pieces(h):
                        for p0, p1 in _quad_split(r0, a0, r1 - r0):
                            nc.vector.tensor_tensor(
                                attnT[t][a0 + p0:a0 + p1, :], pv[r0 + p0:r0 + p1, :],
                                bct[r0 + p0:r0 + p1, :], ALU.mult)
                    pump()
                while pump():
                    pass

            def emit_oproj(jj, m):
                cps = pcp.tile([128, 512], FP, name="op_ps")
                for k in range(6):
                    nc.tensor.matmul(
                        cps[:], lhsT=ow_k[k][:, 128 * m:128 * (m + 1)],
                        rhs=attnTs[jj][k][:], start=(k == 0), stop=(k == 5))
                cev = pce.tile([128, 512], BF, name="op_ev")
                nc.vector.tensor_copy(cev[:], cps[:])
                nc.sync.dma_start(
                    outT[128 * m:128 * (m + 1), 512 * jj:512 * (jj + 1)],
                    cev[:])

            emit_attn_chunk(0, iter([]))
            emit_attn_chunk(1, iter(
                [(lambda m=m: emit_oproj(0, m)) for m in range(24)]))
            emit_attn_chunk(2, iter(
                [(lambda m=m: emit_oproj(1, m)) for m in range(24)]))
            emit_attn_chunk(3, iter(
                [(lambda m=m: emit_oproj(2, m)) for m in range(24)]))
            for m in range(23):
                emit_oproj(3, m)
            cps = pcp.tile([128, 512], FP, name="op_ps")
            for k in range(6):
                nc.tensor.matmul(cps[:], lhsT=ow_k[k][:, 128 * 23:128 * 24],
                                 rhs=attnTs[3][k][:], start=(k == 0), stop=(k == 5))
            cev = pce.tile([128, 512], BF, name="op_ev")
            nc.scalar.copy(cev[0:64, :], cps[0:64, :])
            nc.vector.tensor_copy(cev[64:128, :], cps[64:128, :])
            nc.sync.dma_start(outT[128 * 23:128 * 23 + 64, 512 * 3:512 * 4],
                              cev[0:64, :])
            nc.sync.dma_start(outT[128 * 23 + 64:128 * 24, 512 * 3:512 * 4],
                              cev[64:128, :])

    return nc


_NC_CACHE = [None]


def _host_prep(hidden_states, position_ids, qkv_w, o_w):
    BFn = ml_dtypes.bfloat16
    hidden = np.asarray(hidden_states, dtype=np.float32)
    qkv_w = np.asarray(qkv_w, dtype=np.float32)
    o_w = np.asarray(o_w, dtype=np.float32)
    pos = np.asarray(position_ids)

    inv_freq = 1.0 / (ROPE_THETA ** (np.arange(0, D, 2) / D))  # [48]
    kk = np.arange(128)[:, None]
    uu = np.arange(128)[None, :]
    tri = (kk <= uu).astype(BFn)

    hT = [np.ascontiguousarray(hidden[b].T).astype(BFn) for b in range(B)]
    cosT, sinT = [], []
    for b in range(B):
        ang = pos[b].astype(np.float64)[None, :] * inv_freq[:, None]  # [48,S]
        ang = np.concatenate([ang, ang], 0)                           # [96,S]
        cosT.append(np.cos(ang).astype(BFn))
        st = np.sin(ang).astype(np.float64)
        st[0:48] *= -1.0
        sinT.append(st.astype(BFn))

    in_maps = []
    for c in range(8):
        b, g = c // 4, c % 4
        q_rows = qkv_w[768 * g:768 * (g + 1)]
        k_rows = qkv_w[3072 + 192 * g:3072 + 192 * (g + 1)]
        v_rows = qkv_w[3840 + 192 * g:3840 + 192 * (g + 1)]
        wqkvT = np.ascontiguousarray(
            np.concatenate([q_rows, k_rows, v_rows], 0).T).astype(BFn)
        o_wT = np.ascontiguousarray(o_w[:, 768 * g:768 * (g + 1)].T).astype(BFn)
        in_maps.append({
            "hT": hT[b], "wqkvT": wqkvT, "o_wT": o_wT,
            "cosT": cosT[b], "sinT": sinT[b], "tri": tri,
            "ones96": np.ones((1, 96), dtype=np.float32),
        })
    return in_maps


def kernel(**inputs):
    in_maps = _host_prep(**inputs)
    if _NC_CACHE[0] is None:
        _NC_CACHE[0] = _build()
    res = run_bass_kernel_spmd(_NC_CACHE[0], in_maps, list(range(8)))
    out = np.empty((B, S, HID), dtype=np.float32)
    for b in range(B):
        acc = res.results[4 * b]["outT"].astype(np.float32)
        for g in range(1, 4):
            acc = acc + res.results[4 * b + g]["outT"].astype(np.float32)
        out[b] = acc.T
    return out


# revision 41
# speedup vs baseline: 1.4365x; 1.0313x over previous
"""Phi3 attention (B=2, S=2048, HID=3072, H=32, KVH=8, D=96) on 8 trn2
cores. Sharding: core c -> batch b=c//4, head-group g=c%4 (8 q-heads,
2 kv-heads per core). v2: all-bf16 operands, SBUF-resident q/k/v (no
DRAM spill), per-chunk RoPE pipelined into the QKV projection, xbar
DMA v-transpose, causally width-trimmed scores/PV, divide-based
softmax normalization with Pool partition-broadcast, o_proj interleaved
into the next chunk's head loop, direct PSUM->DRAM output DMA; host
sums the 4 group partials per batch."""
import numpy as np
import ml_dtypes
from contextlib import ExitStack

from concourse import bass, mybir, tile
from concourse.bass_utils import run_bass_kernel_spmd
from concourse.vector_clock import ScopedClock

# ---------------------------------------------------------------------------
# Workaround for this walrus build's limit of 1 sync-wait command per
# instruction: split excess waits onto prefix NoOps (same-engine program
# order preserves semantics), and replace the TileContext exit drain
# (which waits on every semaphore at once) with single-wait nops.
_MAX_WAITS = 1
_ws_counter = [0]


def _split_excess_waits(tc, ordered):
    for _bb, insts in ordered.items():
        new_list = []
        for inst in insts:
            si = inst.sync_info
            waits = list(si.on_wait) if si is not None else []
            if len(waits) > _MAX_WAITS and not isinstance(
                inst, (tile.TileBranchInst, tile.BassTileLoopBlock)
            ):
                excess, keep = waits[:-_MAX_WAITS], waits[-_MAX_WAITS:]
                for j in range(0, len(excess), _MAX_WAITS):
                    chunk = excess[j : j + _MAX_WAITS]
                    _ws_counter[0] += 1
                    nop = mybir.InstNoOp(
                        name=f"I-waitsplit-{_ws_counter[0]}", engine=inst.engine
                    )
                    nop.sync_info = mybir.SyncInfo(on_wait=chunk, on_update=[])
                    tc.nc.register_instruction(nop)
                    new_list.append(nop)
                inst.sync_info = mybir.SyncInfo(
                    on_wait=keep, on_update=list(si.on_update)
                )
            new_list.append(inst)
        insts[:] = new_list


_orig_lower = tile.TileContext._lower_ordered_insts


def _lower_with_split(self, ordered):
    _split_excess_waits(self, ordered)
    return _orig_lower(self, ordered)


def _drain_and_barrier(self, tick_clock, wait_clock):
    vc = tick_clock.global_clock
    for i in range(len(vc)):
        if vc[i] == 0:
            continue
        partial = ScopedClock()
        partial.require_at_least(None, i, vc[i])
        nop = self.nc.sync.nop(hint=f"drain_split_{i}", nofuse=True)
        wait_clock.add_sem_waits(nop.ins, partial)

    drain_inst = self.nc.sync.drain()
    wait_clock.add_sem_waits(drain_inst.ins, ScopedClock())

    self.nc.all_engine_barrier()
    assert self.sems is not None
    popped = self.nc._tile_sem_poison_stack.pop()
    assert popped is self._sem_poison
    self.nc.clear_and_free_semaphores(list(self.sems.allocated().values()))
    self.nc.all_engine_barrier()


def _install_patch():
    tile.TileContext._drain_and_barrier = _drain_and_barrier
    tile.TileContext._lower_ordered_insts = _lower_with_split


_install_patch()
# ---------------------------------------------------------------------------

FP = mybir.dt.float32
FR = mybir.dt.float32r
BF = mybir.dt.bfloat16
AF = mybir.ActivationFunctionType
ALU = mybir.AluOpType

H, KVH, D, HID = 32, 8, 96, 3072
B, S = 2, 2048
ROPE_THETA = 10000.0
SCALE = float(D) ** -0.5
NQ = 8      # q heads per core
NKV = 2     # kv heads per core
LAG = 6


def _qkv_evac_pieces(m):
    """Evacuation pieces of phase-A psum m-tile m (rows 128m..128m+128 of
    the 1152 qkv rows) -> list of (psum_r0, psum_r1, dest_kind, dest_idx,
    dest_r0). dest_kind: 0=q head, 1=k head, 2=v head."""
    out = []
    lo, hi = 128 * m, 128 * (m + 1)
    bounds = [(0, 768, 0, 96), (768, 960, 1, 96), (960, 1152, 2, 96)]
    r = lo
    while r < hi:
        for s0, s1, kind, hd in bounds:
            if s0 <= r < s1:
                idx = (r - s0) // hd
                within = (r - s0) % hd
                span = min(hi - r, hd - within, s1 - r)
                out.append((r - lo, r - lo + span, kind, idx, within))
                r += span
                break
    return out


def _quad_split(a, b, span):
    """Split [0, span) into offsets valid for partition ranges starting at
    a+off and b+off: a range starting at partition p may span at most its
    buddy block (128 at 0, 64 at 64, 32 at 32/96)."""
    def blk(x):
        x %= 128
        return 128 if x == 0 else (x & -x)
    out = []
    p = 0
    while p < span:
        step = min(blk(a + p), blk(b + p), span - p)
        out.append((p, p + step))
        p += step
    return out


def _norm_pieces(h):
    """Normalize pieces for head h: (pv_r0, pv_r1, attnT_tile, attnT_r0)."""
    out = []
    g0 = 96 * h
    r = 0
    while r < 96:
        t, within = (g0 + r) // 128, (g0 + r) % 128
        span = min(96 - r, 128 - within)
        out.append((r, r + span, t, within))
        r += span
    return out


def _build():
    nc = bass.Bass()
    hT = nc.declare_dram_parameter("hT", [HID, S], BF, isOutput=False)
    wqkvT = nc.declare_dram_parameter("wqkvT", [HID, 1152], BF, isOutput=False)
    o_wT = nc.declare_dram_parameter("o_wT", [768, HID], BF, isOutput=False)
    cosT_d = nc.declare_dram_parameter("cosT", [96, S], BF, isOutput=False)
    sinT_d = nc.declare_dram_parameter("sinT", [96, S], BF, isOutput=False)
    tri_d = nc.declare_dram_parameter("tri", [128, 128], BF, isOutput=False)
    ones_d = nc.declare_dram_parameter("ones96", [1, 96], FR, isOutput=False)
    outT = nc.declare_dram_parameter("outT", [HID, S], BF, isOutput=True)

    with tile.TileContext(nc) as tc, ExitStack() as ctx:
        pers = ctx.enter_context(tc.tile_pool(name="pers", bufs=1))
        tri_t = pers.tile([128, 128], BF)
        cos_t = pers.tile([96, S], BF)
        sin_t = pers.tile([96, S], BF)
        q_t = [pers.tile([96, S], BF, name=f"q{h}") for h in range(NQ)]
        k_t = [pers.tile([96, S], BF, name=f"k{i}") for i in range(NKV)]
        vx_t = [pers.tile([128, 16 * 97], BF, name=f"vx{i}") for i in range(NKV)]
        ones_t = pers.tile([1, 96], FR, name="ones96")
        for i in range(NKV):
            nc.gpsimd.memset(vx_t[i][:], 1.0)

        # ------------------ interleaved QKV projection + attention ---------
        rr = [0]  # round-robin engine counter for psum-evac copies

        def _copy(eng, dst, src):
            # psum-reading copies: ACT or DVE only (GPSIMD cannot access PSUM)
            if eng % 2 == 0:
                nc.scalar.copy(dst, src)
            else:
                nc.vector.tensor_copy(dst, src)

        attnTs = {}

        # ---------------- Phase A: QKV projection + RoPE + vT --------------
        with tc.tile_pool(name="pa_w", bufs=1) as pw, \
             tc.tile_pool(name="pa_h", bufs=2) as ph, \
             tc.tile_pool(name="pa_rot", bufs=5) as prot, \
             tc.tile_pool(name="pa_vs", bufs=2) as pvs, \
             tc.tile_pool(name="pa_vt", bufs=3) as pvt, \
             tc.tile_pool(name="pa_ps", bufs=5, space="PSUM") as pps:
            w_k = [pw.tile([128, 1152], BF, name=f"w{k}") for k in range(24)]

            def emit_a_dma(n, with_w=False):
                c0 = 512 * n
                ht = [ph.tile([128, 512], BF, name=f"ht{k}") for k in range(24)]
                for k in range(24):
                    if with_w:
                        nc.sync.dma_start(w_k[k][:],
                                          wqkvT[128 * k:128 * (k + 1), :])
                    nc.sync.dma_start(ht[k][:],
                                      hT[128 * k:128 * (k + 1), c0:c0 + 512])
                vst = [pvs.tile([96, 512], BF, name=f"vs{i}")
                       for i in range(NKV)]
                return ht, vst

            def a_mtiles(n, ht, vst, group):
                c0 = 512 * n
                pss = {m: pps.tile([128, 512], FP, name="pa_ps")
                       for m in group}
                for k in range(24):
                    for m in group:
                        nc.tensor.matmul(
                            pss[m][:], lhsT=w_k[k][:, 128 * m:128 * (m + 1)],
                            rhs=ht[k][:], start=(k == 0), stop=(k == 23))
                for m in group:
                    for r0, r1, kind, idx, d0 in _qkv_evac_pieces(m):
                        tgt = (q_t, k_t, None)[kind]
                        for p0, p1 in _quad_split(r0, d0, r1 - r0):
                            if kind == 2:
                                dst = vst[idx][d0 + p0:d0 + p1, 0:512]
                            else:
                                dst = tgt[idx][d0 + p0:d0 + p1, c0:c0 + 512]
                            nc.scalar.copy(dst, pss[m][r0 + p0:r0 + p1, :])

            def a_rope_vt(n, vst):
                # rot staging via the ACT DMA queue so the bulk SP queue is
                # never gated by the rope dependency chain; k tensors first
                c0 = 512 * n
                for t in [NQ, NQ + 1] + list(range(NQ)):
                    src = q_t[t] if t < NQ else k_t[t - NQ]
                    rot = prot.tile([96, 512], BF, name="rot")
                    nc.sync.dma_start(rot[0:48, :], src[48:96, c0:c0 + 512])
                    nc.sync.dma_start(rot[48:96, :], src[0:48, c0:c0 + 512])
                    nc.vector.tensor_tensor(rot[:], rot[:],
                                            sin_t[:, c0:c0 + 512], ALU.mult)
                    nc.vector.tensor_tensor(src[:, c0:c0 + 512],
                                            src[:, c0:c0 + 512],
                                            cos_t[:, c0:c0 + 512], ALU.mult)
                    nc.vector.tensor_tensor(src[:, c0:c0 + 512],
                                            src[:, c0:c0 + 512], rot[:], ALU.add)
                for i in range(NKV):
                    for t in range(4):
                        kt = 4 * n + t
                        vt = pvt.tile([128, 96], BF, name="vt")
                        nc.sync.dma_start_transpose(
                            vt[:], vst[i][:, 128 * t:128 * (t + 1)])
                        nc.gpsimd.tensor_copy(
                            vx_t[i][:, 97 * kt:97 * kt + 96], vt[:])

            ht0, vst0 = emit_a_dma(0, with_w=True)
            nc.sync.dma_start(tri_t[:], tri_d[:])
            nc.sync.dma_start(cos_t[:], cosT_d[:])
            nc.sync.dma_start(sin_t[:], sinT_d[:])
            nc.sync.dma_start(ones_t[:], ones_d[:])
            ht1, vst1 = emit_a_dma(1)
            GROUPS = ([0, 2], [2, 4], [4, 6], [6, 8], [8, 9])
            for group in GROUPS:
                a_mtiles(0, ht0, vst0, range(*group))
            ht2, vst2 = emit_a_dma(2)
            a_rope_vt(0, vst0)
            for group in GROUPS:
                a_mtiles(1, ht1, vst1, range(*group))
            ht3, vst3 = emit_a_dma(3)
            a_rope_vt(1, vst1)
            for group in GROUPS:
                a_mtiles(2, ht2, vst2, range(*group))
            a_rope_vt(2, vst2)
            for group in GROUPS:
                a_mtiles(3, ht3, vst3, range(*group))
            a_rope_vt(3, vst3)

        # ---------------- Phase B: attention + o_proj ----------------------
        with tc.tile_pool(name="pb_an", bufs=2) as pan, \
             tc.tile_pool(name="pb_et", bufs=8) as pet, \
             tc.tile_pool(name="pb_d", bufs=3) as pbd, \
             tc.tile_pool(name="pb_ow", bufs=1) as pow_, \
             tc.tile_pool(name="pb_ce", bufs=3) as pce, \
             tc.tile_pool(name="pb_ss", bufs=3, space="PSUM") as psc, \
             tc.tile_pool(name="pb_pv", bufs=2, space="PSUM") as ppv, \
             tc.tile_pool(name="pb_bc", bufs=1, space="PSUM") as pbc, \
             tc.tile_pool(name="pb_op", bufs=2, space="PSUM") as pcp:
            ow_k = [pow_.tile([128, HID], BF, name=f"ow{k}") for k in range(6)]
            for k in range(6):
                nc.sync.dma_start(ow_k[k][:], o_wT[128 * k:128 * (k + 1), :])

            def emit_attn_chunk(j, backfill):
                """Emit attention chunk j; pull PE backfill thunks from the
                iterator at pipeline bubbles."""
                def pump(k=1):
                    for _ in range(k):
                        t = next(backfill, None)
                        if t is None:
                            return False
                        t()
                    return True

                q0 = 512 * j
                n_kt = 4 * j + 4
                attnT = [pan.tile([128, 512], BF, name=f"anT{t}") for t in range(6)]
                attnTs[j] = attnT
                for h in range(NQ):
                    kv = h // 4
                    pv = ppv.tile([97, 512], FP, name="pv_ps")
                    ets = [None] * n_kt
                    for step in range(n_kt + LAG):
                        if step < n_kt:
                            kt = step
                            r = kt - 4 * j
                            off = 128 * r if r > 0 else 0
                            w = 512 - off
                            ss = psc.tile([128, 512], FP, name="ss_ps")
                            nc.tensor.matmul(
                                ss[:, 0:w], lhsT=k_t[kv][:, 128 * kt:128 * (kt + 1)],
                                rhs=q_t[h][:, q0 + off:q0 + 512],
                                start=True, stop=True)
                            et = pet.tile([128, 512], BF, name="et")
                            nc.scalar.activation(et[:, 0:w], ss[:, 0:w], AF.Exp,
                                                 scale=SCALE)
                            if r >= 0:  # zero the causal triangle
                                eng = nc.vector if h % 2 == 0 else nc.gpsimd
                                eng.tensor_tensor(et[:, 0:128], et[:, 0:128],
                                                  tri_t[:], ALU.mult)
                            ets[kt] = (et, off, w)
                        if step >= LAG:
                            kt2 = step - LAG
                            et, off, w = ets[kt2]
                            nc.tensor.matmul(
                                pv[:, off:off + w],
                                lhsT=vx_t[kv][:, 97 * kt2:97 * (kt2 + 1)],
                                rhs=et[:, 0:w], start=(kt2 == 0),
                                stop=(kt2 == n_kt - 1), skip_group_check=True)
                        if step == 3:
                            pump()
                        if step == 3 or (step > 3 and (step - 3) % 4 == 0
                                         and step < n_kt):
                            pump()
                    # normalization: recip(den) -> PE broadcast -> mult,
                    # spread across DVE and ACT to keep either queue short
                    rec = pbd.tile([1, 512], FR, name="rec")
                    with nc.allow_low_precision(reason="f32r stores full f32 bits"):
                        nc.vector.reciprocal(rec[:], pv[96:97, :])
                    bcp = pbc.tile([96, 512], FP, name="bc_ps")
                    nc.tensor.matmul(bcp[:], lhsT=ones_t[:], rhs=rec[:],
                                     start=True, stop=True)
                    for r0, r1, t, a0 in _norm_pieces(h):
                        for p0, p1 in _quad_split(r0, a0, r1 - r0):
                            nc.vector.tensor_tensor(
                                attnT[t][a0 + p0:a0 + p1, :],
                                pv[r0 + p0:r0 + p1, :],
                                bcp[r0 + p0:r0 + p1, :], ALU.mult)
                    pump()
                while pump():
                    pass

            def emit_oproj(jj, m):
                cps = pcp.tile([128, 512], FP, name="op_ps")
                for k in range(6):
                    nc.tensor.matmul(
                        cps[:], lhsT=ow_k[k][:, 128 * m:128 * (m + 1)],
                        rhs=attnTs[jj][k][:], start=(k == 0), stop=(k == 5))
                cev = pce.tile([128, 512], BF, name="op_ev")
                _copy(0 if jj == 0 else m, cev[:], cps[:])
                nc.sync.dma_start(
                    outT[128 * m:128 * (m + 1), 512 * jj:512 * (jj + 1)],
                    cev[:])

            emit_attn_chunk(0, iter([]))
            emit_attn_chunk(1, iter(
                [(lambda m=m: emit_oproj(0, m)) for m in range(24)]))
            emit_attn_chunk(2, iter(
                [(lambda m=m: emit_oproj(1, m)) for m in range(24)]))
            emit_attn_chunk(3, iter(
                [(lambda m=m: emit_oproj(2, m)) for m in range(24)]))
            for m in range(23):
                emit_oproj(3, m)
            cps = pcp.tile([128, 512], FP, name="op_ps")
            for k in range(6):
                nc.tensor.matmul(cps[:], lhsT=ow_k[k][:, 128 * 23:128 * 24],
                                 rhs=attnTs[3][k][:], start=(k == 0), stop=(k == 5))
            cev = pce.tile([128, 512], BF, name="op_ev")
            for qi in range(4):
                r0 = 32 * qi
                eng = nc.scalar if qi % 2 == 0 else nc.vector
                if qi % 2 == 0:
                    nc.scalar.copy(cev[r0:r0 + 32, :], cps[r0:r0 + 32, :])
                else:
                    nc.vector.tensor_copy(cev[r0:r0 + 32, :], cps[r0:r0 + 32, :])
                nc.sync.dma_start(
                    outT[128 * 23 + r0:128 * 23 + r0 + 32, 512 * 3:512 * 4],
                    cev[r0:r0 + 32, :])

    return nc


_NC_CACHE = [None]


def _host_prep(hidden_states, position_ids, qkv_w, o_w):
    BFn = ml_dtypes.bfloat16
    hidden = np.asarray(hidden_states, dtype=np.float32)
    qkv_w = np.asarray(qkv_w, dtype=np.float32)
    o_w = np.asarray(o_w, dtype=np.float32)
    pos = np.asarray(position_ids)

    inv_freq = 1.0 / (ROPE_THETA ** (np.arange(0, D, 2) / D))  # [48]
    kk = np.arange(128)[:, None]
    uu = np.arange(128)[None, :]
    tri = (kk <= uu).astype(BFn)

    hT = [np.ascontiguousarray(hidden[b].T).astype(BFn) for b in range(B)]
    cosT, sinT = [], []
    for b in range(B):
        ang = pos[b].astype(np.float64)[None, :] * inv_freq[:, None]  # [48,S]
        ang = np.concatenate([ang, ang], 0)                           # [96,S]
        cosT.append(np.cos(ang).astype(BFn))
        st = np.sin(ang).astype(np.float64)
        st[0:48] *= -1.0
        sinT.append(st.astype(BFn))

    in_maps = []
    for c in range(8):
        b, g = c // 4, c % 4
        q_rows = qkv_w[768 * g:768 * (g + 1)]
        k_rows = qkv_w[3072 + 192 * g:3072 + 192 * (g + 1)]
        v_rows = qkv_w[3840 + 192 * g:3840 + 192 * (g + 1)]
        wqkvT = np.ascontiguousarray(
            np.concatenate([q_rows, k_rows, v_rows], 0).T).astype(BFn)
        o_wT = np.ascontiguousarray(o_w[:, 768 * g:768 * (g + 1)].T).astype(BFn)
        in_maps.append({
            "hT": hT[b], "wqkvT": wqkvT, "o_wT": o_wT,
            "cosT": cosT[b], "sinT": sinT[b], "tri": tri,
            "ones96": np.ones((1, 96), dtype=np.float32),
        })
    return in_maps


def kernel(**inputs):
    in_maps = _host_prep(**inputs)
    if _NC_CACHE[0] is None:
        _NC_CACHE[0] = _build()
    res = run_bass_kernel_spmd(_NC_CACHE[0], in_maps, list(range(8)))
    out = np.empty((B, S, HID), dtype=np.float32)
    for b in range(B):
        acc = res.results[4 * b]["outT"].astype(np.float32)
        for g in range(1, 4):
            acc = acc + res.results[4 * b + g]["outT"].astype(np.float32)
        out[b] = acc.T
    return out
